# revision 38
# baseline (speedup 1.0000x reference)
"""Multi-head attention (B=2, S=2048, D=1024, H=16) on 8 TRN2 NeuronCores.

Sharding: (batch, head-group) SPMD. Core c handles batch b = c//4 and local
heads [4*(c%4), 4*(c%4)+4). Each core computes its 4 heads' attention plus the
partial o-projection (row-parallel over the head dimension); the host sums the
4 partial outputs per batch and adds b_o.

Schedule (derived from per-instruction NTFF traces; 370us -> 268us):
  - phase-1 projection quarters interleaved into qb0's attention stream
  - tensor queue ordered S(i) ... PV(i-2) so the PE never micro-idles
    (HAM clock-gate throttles to 1.2GHz after ~3.4us of PE idle)
  - ACT runs Exp only (the engine is the ~1us/tile bottleneck); the softmax
    denominator reciprocal runs on DVE after a DRAM round-trip reshape to
    [128,8] lanes, and 1/den is broadcast back via a step-0-AP DMA
  - batched, demand-ordered DMAs (one per weight tensor / x quarter /
    4-mask group) to kill the ~615ns-per-issue ramp serialization
  - boundary work (cnr copy, dance, cn muls, o_proj chunks) hooked into the
    next q-block's stream late enough that it never blocks S matmuls or the
    DVE mask-mul FIFO
  - last qb runs pair-major so its first half-boundary overlaps compute;
    dependency-free dummy matmuls keep the PE warm through the final dance
"""
import os
import sys

if "/opt/trn_rl_repo" not in sys.path:
    sys.path.insert(0, "/opt/trn_rl_repo")
os.environ.setdefault("JAX_PLATFORMS", "axon,cpu")

from contextlib import ExitStack

import ml_dtypes
import numpy as np

import concourse.bass as bass
import concourse.tile as tile
from concourse import bacc, library_config, mybir
from concourse.bass_utils import run_bass_kernel_spmd

F32 = mybir.dt.float32
BF16 = mybir.dt.bfloat16
EXP = mybir.ActivationFunctionType.Exp

B, S, D = 2, 2048, 1024
H, HD = 16, 64
HL = 4            # local heads per core
CH = HL * HD      # 256 local channels
N_CORES = 8
KC = D // 128     # 8 contraction chunks for the projections
NKT = S // 128    # 16 k tiles
NIT = NKT * 2     # 32 (ktile, pair) iterations per q block
LAG = 2           # PV trails S by this many iterations in the tensor queue

_CACHE = {}


def _build_nc():
    nc = bacc.Bacc("TRN2", target_bir_lowering=False)
    xT_d = nc.declare_dram_parameter("xT", [D, S], BF16, isOutput=False)
    mk_d = nc.declare_dram_parameter("maskT", [S, S], BF16, isOutput=False)
    wqT_d = nc.declare_dram_parameter("wqT", [D, CH], BF16, isOutput=False)
    wkT_d = nc.declare_dram_parameter("wkT", [D, CH], BF16, isOutput=False)
    wvT_d = nc.declare_dram_parameter("wvT", [D, CH], BF16, isOutput=False)
    woT_d = nc.declare_dram_parameter("woT", [CH, D], BF16, isOutput=False)
    yT_d = nc.declare_dram_parameter("yT", [D, S], F32, isOutput=True)
    # scratch for the denominator reshape dance, per (qb, pair-half) slot
    scr_d = nc.declare_dram_parameter("scr", [8, 2, 1024], F32, isOutput=True)

    with tile.TileContext(nc) as tc, ExitStack() as ctx:
        nc.gpsimd.load_library(library_config.attn)
        const = ctx.enter_context(tc.tile_pool(name="const", bufs=1))
        work = ctx.enter_context(tc.tile_pool(name="work", bufs=1))
        psum = ctx.enter_context(tc.tile_pool(name="psum", bufs=1, space="PSUM"))

        # ---- resident tensors (batched DMAs, demand-ordered across two
        # queues: sync carries weights+mask, gpsimd carries x quarters) ----
        mk4 = [const.tile([128, 4, S], BF16, name=f"mk{g}") for g in range(4)]
        qt = [const.tile([128, S], BF16, name=f"qt{i}") for i in range(2)]
        kt_sb = [const.tile([128, S], BF16, name=f"kt{i}") for i in range(2)]
        v_sb = [const.tile([128, HL * 65], BF16, name=f"v{i}") for i in range(NKT)]
        for st in range(NKT):
            nc.gpsimd.memset(
                v_sb[st].rearrange("p (h c) -> p h c", h=HL)[:, :, 64:65], 1.0
            )
        w_all = {}
        for nm in ("wk", "wv", "wq"):
            w_all[nm] = const.tile([128, KC, CH], BF16, name=nm)
        wo_all = const.tile([64, HL, D], BF16, name="wo")
        w_dram = {"wk": wkT_d, "wv": wvT_d, "wq": wqT_d}

        def load_w(nm):
            nc.sync.dma_start(
                w_all[nm][:], w_dram[nm].rearrange("(k p) c -> p k c", p=128)
            )

        load_w("wk")  # first on the queue: kt jobs gate the whole pipeline

        def mask_ap(kt, qb):
            return mk4[kt // 4][:, kt % 4, qb * 512:(qb + 1) * 512]

        # ---- phase-1 quarter: KT/V/QT projections for seq quarter qh ----
        xts = {}

        def load_xt(qh):
            xt = work.tile([128, KC, 512], BF16, name="xt", tag="xt", bufs=2)
            nc.sync.dma_start(
                xt[:],
                xT_d[:, qh * 512:(qh + 1) * 512].rearrange("(k p) s -> p k s", p=128),
            )
            xts[qh] = xt

        def load_mk(qh):
            nc.sync.dma_start(
                mk4[qh][:],
                mk_d[qh * 512:(qh + 1) * 512, :].rearrange("(j p) s -> p j s", p=128),
            )

        def load_quarter(qh):
            load_xt(qh)
            load_mk(qh)

        def quarter_qk(qh, wsb, dsts):
            xt = xts[qh]
            ps = psum.tile([128, 1024], F32, name="p1qk", tag="psa", bufs=2)
            for mt in range(2):
                for k in range(KC):
                    nc.tensor.matmul(
                        ps[:, mt * 512:(mt + 1) * 512],
                        wsb[:, k, mt * 128:(mt + 1) * 128],
                        xt[:, k, :],
                        start=(k == 0), stop=(k == KC - 1),
                    )
            for mt in range(2):
                nc.scalar.copy(
                    dsts[mt][:, qh * 512:(qh + 1) * 512],
                    ps[:, mt * 512:(mt + 1) * 512],
                )

        def quarter_v(qh):
            xt = xts[qh]
            vp = psum.tile([128, 1024], F32, name="p1v", tag="psa", bufs=2)
            for sl in range(4):
                for k in range(KC):
                    nc.tensor.matmul(
                        vp[:, sl * 256:(sl + 1) * 256],
                        xt[:, k, sl * 128:(sl + 1) * 128],
                        w_all["wv"][:, k, :],
                        start=(k == 0), stop=(k == KC - 1),
                    )
            for sl in range(4):
                st = qh * 4 + sl
                nc.scalar.copy(
                    v_sb[st].rearrange("p (h c) -> p h c", h=HL)[:, :, 0:64],
                    vp[:, sl * 256:(sl + 1) * 256].rearrange(
                        "p (h c) -> p h c", h=HL
                    ),
                )

        def quarter_tail(qh):
            if qh < 3:
                load_quarter(qh + 1)

        # ---- half-boundary: reciprocal of one pair's denominators ----
        # slot = qb*2 + pair; processes cq columns [pair*1024, pair*1024+1024)
        bnd = {}

        def dance(slot, cq, pair):
            c0 = pair * 1024
            cnr = work.tile([65, 1024], F32, name=f"cnr{pair}", tag=f"cnr{pair}",
                            bufs=2)
            nc.scalar.copy(cnr[:], cq[0:65, c0:c0 + 1024])
            nc.sync.dma_start(
                scr_d[slot, 0, :].rearrange("(a c) -> a c", a=1), cnr[64:65, :]
            )
            r128 = work.tile([128, 8], F32, name="r128", tag="r128", bufs=2)
            nc.sync.dma_start(
                r128[:], scr_d[slot, 0, :].rearrange("(p c) -> p c", p=128)
            )
            nc.vector.reciprocal(r128[:], r128[:])
            nc.sync.dma_start(
                scr_d[slot, 1, :].rearrange("(p c) -> p c", p=128), r128[:]
            )
            rb = work.tile([64, 1024], F32, name=f"rb{pair}", tag=f"rb{pair}",
                           bufs=2)
            scr1 = scr_d[slot, 1, :]
            nc.sync.dma_start(
                rb[:], bass.AP(scr1.tensor, scr1.offset, [(0, 64), (1, 1024)])
            )
            bnd[("cnr", slot)] = cnr
            bnd[("rb", slot)] = rb

        def cn_mul(slot, pair):
            cn = bnd.get(("cn", slot // 2))
            if cn is None:
                cn = work.tile([64, 2048], BF16, name="cn", tag="cn", bufs=2)
                bnd[("cn", slot // 2)] = cn
            c0 = pair * 1024
            nc.vector.tensor_mul(
                cn[:, c0:c0 + 1024],
                bnd[("cnr", slot)][0:64, :],
                bnd[("rb", slot)][:],
            )

        op_live = {}

        def op_slice(qb, ot):
            # one self-contained [128, 512] o_proj output tile: short tensor-
            # queue occupancy so the S matmuls feeding ACT are never blocked
            cn = bnd[("cn", qb)]
            op = psum.tile([128, 512], F32, name="op", tag="psa", bufs=2)
            for h in range(HL):
                nc.tensor.matmul(
                    op[:, 0:512],
                    wo_all[:, h, ot * 128:(ot + 1) * 128],
                    cn[:, h * 512:(h + 1) * 512],
                    start=(h == 0), stop=(h == HL - 1),
                )
            ysb = work.tile([128, 512], F32, name="ysb", tag="ysb", bufs=2)
            nc.vector.tensor_copy(ysb[:], op[:, 0:512])
            nc.sync.dma_start(
                yT_d[ot * 128:(ot + 1) * 128, qb * 512:(qb + 1) * 512],
                ysb[:],
            )

        def op_chunk(qb, g, half=None):
            # half=None: both head pairs; half=0/1: only that pair's
            # contraction (accumulated across two calls)
            cn = bnd[("cn", qb)]
            if half in (None, 0):
                op = psum.tile([128, 1024], F32, name="op", tag="psa", bufs=2)
                op_live[(qb, g)] = op
            else:
                op = op_live[(qb, g)]
            hs = range(HL) if half is None else range(half * 2, half * 2 + 2)
            for j in range(2):
                ot = g * 2 + j
                for h in hs:
                    nc.tensor.matmul(
                        op[:, j * 512:(j + 1) * 512],
                        wo_all[:, h, ot * 128:(ot + 1) * 128],
                        cn[:, h * 512:(h + 1) * 512],
                        start=(h == hs[0] if half != 1 else False),
                        stop=(h == hs[-1] if half != 0 else False),
                    )
            if half == 0:
                return
            ysb = work.tile([128, 1024], F32, name="ysb", tag="ysb", bufs=2)
            if qb == 3:
                nc.scalar.copy(ysb[:], op[:])  # ACT is idle in the tail
            else:
                nc.vector.tensor_copy(ysb[:], op[:])
            nc.sync.dma_start(
                yT_d[g * 256:(g + 1) * 256,
                     qb * 512:(qb + 1) * 512].rearrange("(o r) c -> r o c", o=2),
                ysb.rearrange("r (o c) -> r o c", o=2),
            )

        # ---- pipelined attention over one q block ----
        def attention_qb(qb, cq, hooks, pair_major=False):
            if pair_major:
                order = [(kt, p) for p in range(2) for kt in range(NKT)]
            else:
                order = [(kt, p) for kt in range(NKT) for p in range(2)]
            pend = []
            for i in range(NIT + LAG):
                for f in hooks.get(i, []):
                    f()
                if i < NIT:
                    kt, pair = order[i]
                    tq = psum.tile([128, 1024], F32, name="tq", tag="psa", bufs=2)
                    for hh in range(2):
                        nc.tensor.matmul(
                            tq[:, hh * 512:(hh + 1) * 512],
                            kt_sb[pair][hh * 64:(hh + 1) * 64,
                                        kt * 128:(kt + 1) * 128],
                            qt[pair][hh * 64:(hh + 1) * 64,
                                     qb * 512:(qb + 1) * 512],
                            start=True, stop=True,
                        )
                    ex = work.tile([128, 1024], BF16, name="ex", tag="ex", bufs=4)
                    nc.scalar.activation(ex[:], tq[:], EXP)
                    pt = work.tile([128, 1024], BF16, name="pt", tag="pt", bufs=6)
                    for hh in range(2):
                        nc.vector.tensor_mul(
                            pt[:, hh * 512:(hh + 1) * 512],
                            ex[:, hh * 512:(hh + 1) * 512],
                            mask_ap(kt, qb),
                        )
                    pend.append((kt, pair, pt))
                if i >= LAG:
                    kt, pair, pt = pend.pop(0)
                    for hh in range(2):
                        h = pair * 2 + hh
                        nc.tensor.matmul(
                            cq[0:65, h * 512:(h + 1) * 512],
                            v_sb[kt][:, h * 65:h * 65 + 65],
                            pt[:, hh * 512:(hh + 1) * 512],
                            start=(kt == 0), stop=(kt == NKT - 1),
                        )

        def boundary_hooks(pq, pcq):
            # full boundary for q-block pq, interleaved into the next block;
            # cn_mul sits late enough that the rb DMA dance (~7.5us) is done
            # before it enters the DVE FIFO
            return {
                0: [lambda: dance(pq * 2, pcq, 0),
                    lambda: dance(pq * 2 + 1, pcq, 1)],
                12: [lambda: cn_mul(pq * 2, 0), lambda: cn_mul(pq * 2 + 1, 1)],
                14: [lambda: op_slice(pq, 0)],
                16: [lambda: op_slice(pq, 1)],
                18: [lambda: op_slice(pq, 2)],
                20: [lambda: op_slice(pq, 3)],
                22: [lambda: op_slice(pq, 4)],
                24: [lambda: op_slice(pq, 5)],
                26: [lambda: op_slice(pq, 6)],
                28: [lambda: op_slice(pq, 7)],
            }

        prev_cq = None
        for qb in range(4):
            cq = psum.tile([128, 2048], F32, name="cq", tag="cq", bufs=1)
            if qb == 0:
                # ramp: demand-ordered DMAs — wk (const section), x quarter 0,
                # wv/wq, then masks/next quarter/wo trickle behind
                hooks = {
                    0: [lambda: load_xt(0),
                        lambda: load_w("wv"),
                        lambda: load_w("wq"),
                        lambda: quarter_qk(0, w_all["wk"], kt_sb),
                        lambda: quarter_v(0),
                        lambda: quarter_qk(0, w_all["wq"], qt),
                        lambda: load_mk(0),
                        lambda: quarter_tail(0)],
                    2: [lambda: nc.sync.dma_start(
                            wo_all[:],
                            woT_d.rearrange("(h p) c -> p h c", p=64))],
                }
                for j in (1, 2, 3):
                    hooks[8 * j] = [lambda j=j: quarter_qk(j, w_all["wk"], kt_sb)]
                    hooks[8 * j + 2] = [lambda j=j: quarter_v(j)]
                    hooks[8 * j + 4] = [lambda j=j: quarter_qk(j, w_all["wq"], qt),
                                        lambda j=j: quarter_tail(j)]
            else:
                hooks = boundary_hooks(qb - 1, prev_cq)
            if qb == 3:
                # pair-major: pair 0 finishes at i=15 so its half-boundary
                # overlaps pair 1's compute
                hooks.setdefault(18, []).append(lambda cq=cq: dance(6, cq, 0))
                hooks.setdefault(28, []).append(lambda: cn_mul(6, 0))
            attention_qb(qb, cq, hooks, pair_major=(qb == 3))
            prev_cq = cq

        # tail: qb3's second half-boundary; pair-0 o_proj halves run during
        # the dance to keep the PE warm, pair-1 halves after cn
        dance(7, prev_cq, 1)
        op_chunk(3, 0, half=0)
        op_chunk(3, 1, half=0)
        cn_mul(7, 1)
        # dependency-free matmuls bridge the ~10us dance latency so the HAM
        # clock gate stays at full rate for the final o_proj chunks
        dummy = psum.tile([128, 2048], F32, name="dummy", tag="cq", bufs=1)
        for _ in range(44):
            nc.tensor.matmul(
                dummy[:, 0:512], kt_sb[0][0:64, 0:128], qt[0][0:64, 0:512],
                start=True, stop=True,
            )
        op_chunk(3, 0, half=1)
        op_chunk(3, 1, half=1)
        op_chunk(3, 2)
        op_chunk(3, 3)

    nc.compile()
    return nc


def _get_nc():
    if "nc" not in _CACHE:
        _CACHE["nc"] = _build_nc()
    return _CACHE["nc"]


def kernel(x, mask, w_qkv, b_qkv, w_o, b_o):
    x = np.asarray(x, dtype=np.float32)
    mask = np.asarray(mask)
    w_qkv = np.asarray(w_qkv, dtype=np.float32)
    b_qkv = np.asarray(b_qkv, dtype=np.float32)
    w_o = np.asarray(w_o, dtype=np.float32)
    b_o = np.asarray(b_o, dtype=np.float32)
    assert not b_qkv.any(), "kernel specialized for zero qkv bias"

    scale = np.float32(1.0 / np.sqrt(HD))
    maskT = np.ascontiguousarray(mask.reshape(S, S).T).astype(ml_dtypes.bfloat16)

    w3 = w_qkv.reshape(H, 3, HD, D)  # [head, (q,k,v), hd, D]
    in_maps = []
    for c in range(N_CORES):
        b = c // 4
        h0 = (c % 4) * HL
        heads = list(range(h0, h0 + HL))
        wq = w3[heads, 0].reshape(CH, D) * scale
        wk = w3[heads, 1].reshape(CH, D)
        wv = w3[heads, 2].reshape(CH, D)
        wo_cols = np.concatenate([w_o[:, h * HD:(h + 1) * HD] for h in heads], axis=1)
        in_maps.append({
            "xT": np.ascontiguousarray(x[b].T).astype(ml_dtypes.bfloat16),
            "maskT": maskT,
            "wqT": np.ascontiguousarray(wq.T).astype(ml_dtypes.bfloat16),
            "wkT": np.ascontiguousarray(wk.T).astype(ml_dtypes.bfloat16),
            "wvT": np.ascontiguousarray(wv.T).astype(ml_dtypes.bfloat16),
            "woT": np.ascontiguousarray(wo_cols.T).astype(ml_dtypes.bfloat16),
        })

    nc = _get_nc()
    trace = bool(int(os.environ.get("MHA_TRACE", "0")))
    res = run_bass_kernel_spmd(nc, in_maps, core_ids=list(range(N_CORES)),
                               trace=trace)
    _CACHE["last_results"] = res

    y = np.zeros((B, S, D), dtype=np.float32)
    for c in range(N_CORES):
        y[c // 4] += res.results[c]["yT"].T
    y += b_o
    return y
